# revision 38
# baseline (speedup 1.0000x reference)
"""MoE grouped w8a8 block-quant GEMM + gated combine for 8 Trainium2 cores.

Sharding (expert-parallel, per the hint): core c owns experts [4c,4c+4),
their routed rows [c*16384,(c+1)*16384) (uniform token_count=4096), and
tokens [c*2048,(c+1)*2048). Routed rows are dispatched to their owning
core on the host (the all-to-all dispatch step): x rows are packed
pre-transposed into a K-on-partitions layout, so the device reads them
with fat sequential DMAs. All arithmetic (gate normalization/masking,
scale products, dequant, GEMM, combine) runs on device.

Device pipeline per core:
  Phase A (once): normalize+mask gates and multiply by gathered x-scales
    in a kb-major replicated [128, 512] layout -> xsg written to DRAM
    as a flat [4, RPC] table (row kb holds xs[row,kb]*gate[row]).
  Phase B (once): dequant int8 weights -> bf16 on the scalar engine.
  Main loop, per chunk of R=1024 routed rows (= 128 tokens):
    1. Sequential DMA pulls the pre-dispatched x chunk [128, 2, 2048] int8.
    2. A replicated-read DMA broadcasts xsg rows to all 128 partitions
       (S[p, c, i] = xsg[2c + p//64, ch*R+i]) -- no matmul, no PSUM.
    3. One DVE op dequantizes: xdq = int8 * S (bf16).
    4. 32 matmuls accumulate the gated expert GEMM into [128 tokens, 512]
       PSUM (gates are folded into S, so top-k combine == accumulation).
    5. DVE adds shared_output, writes bf16; DMA out.
"""

import numpy as np
import ml_dtypes

T, TOPK, K, N, E, B = 16384, 8, 512, 512, 32, 128
ROWS = T * TOPK
NCORES = 8
EL = E // NCORES            # experts per core
RPC = ROWS // NCORES        # routed rows per core
TPC = T // NCORES           # tokens per core
R = 1024                    # rows per chunk
NCH = RPC // R              # chunks per core

_cache = {}


def _build(n_chunks=NCH):
    from contextlib import ExitStack
    import concourse.bass as bass
    import concourse.bacc as bacc
    import concourse.tile as tile
    from concourse import mybir

    dt = mybir.dt
    nc = bacc.Bacc("TRN2", target_bir_lowering=False, debug=False,
                   enable_asserts=False)

    xg = nc.dram_tensor("xg", (NCH, 128, 2, 2048), dt.int8, kind="ExternalInput")
    wq = nc.dram_tensor("wq", (EL, 4, 128, 512), dt.int8, kind="ExternalInput")
    gates = nc.dram_tensor("gates", (128, 512), dt.float32, kind="ExternalInput")
    srcdst = nc.dram_tensor("srcdst", (128, 512), dt.int32, kind="ExternalInput")
    xsq = nc.dram_tensor("xsq", (128, 512), dt.bfloat16, kind="ExternalInput")
    # mw bytes: [0:512) msl bf16 (row-tile replicated), [512:640) wscol f32
    mw = nc.dram_tensor("mw", (128, 640), dt.int8, kind="ExternalInput")
    shared = nc.dram_tensor("shared", (TPC, N), dt.bfloat16, kind="ExternalInput")
    out = nc.dram_tensor("out", (TPC, N), dt.bfloat16, kind="ExternalOutput")
    xsgd = nc.dram_tensor("xsgd", (128, 512), dt.bfloat16, kind="Internal")

    AX = mybir.AxisListType
    OP = mybir.AluOpType

    with tile.TileContext(nc) as tc, ExitStack() as ctx:
        const = ctx.enter_context(tc.tile_pool(name="const", bufs=1))
        wraw_p = ctx.enter_context(tc.tile_pool(name="wraw", bufs=2))
        gat_p = ctx.enter_context(tc.tile_pool(name="gat", bufs=5))
        gt_p = ctx.enter_context(tc.tile_pool(name="gtp", bufs=5))
        ssb_p = ctx.enter_context(tc.tile_pool(name="ssb", bufs=4))
        xdq_p = ctx.enter_context(tc.tile_pool(name="xdq", bufs=4))
        sh_p = ctx.enter_context(tc.tile_pool(name="shp", bufs=3))
        ob_p = ctx.enter_context(tc.tile_pool(name="obp", bufs=3))
        sps_p = ctx.enter_context(tc.tile_pool(name="sps", bufs=1, space="PSUM"))
        ops_p = ctx.enter_context(tc.tile_pool(name="ops", bufs=2, space="PSUM"))

        # ---- phase A: xsg = normalized+masked gates * x-scales (kb-major)
        gsb = const.tile([128, 512], dt.float32)
        nc.sync.dma_start(gsb[:], gates.ap())
        ssb = const.tile([128, 512], dt.int32)
        nc.sync.dma_start(ssb[:], srcdst.ap())
        xsb = const.tile([128, 512], dt.bfloat16)
        nc.sync.dma_start(xsb[:], xsq.ap())
        mwt = const.tile([128, 640], dt.int8)
        nc.sync.dma_start(mwt[:], mw.ap())
        msl = mwt[:].bitcast(dt.bfloat16)[:, 0:256]
        wsc = mwt[:].bitcast(dt.float32)[:, 128:160]
        g3 = gsb[:].rearrange("p (t j) -> p t j", j=8)
        sums = const.tile([128, 64], dt.float32)
        nc.vector.tensor_reduce(sums[:], g3, AX.X, OP.add)
        nc.vector.tensor_scalar_max(sums[:], sums[:], 1e-12)
        rec = const.tile([128, 64], dt.float32)
        nc.vector.reciprocal(rec[:], sums[:])
        gn = const.tile([128, 512], dt.float32)
        nc.vector.scalar_tensor_tensor(
            gn[:].rearrange("p (t j) -> p t j", j=8), g3, 1.0,
            rec[:].unsqueeze(2).broadcast_to([128, 64, 8]), OP.mult, OP.mult)
        gm = const.tile([128, 512], dt.bfloat16)
        nc.vector.scalar_tensor_tensor(gm[:], ssb[:], -1, gn[:],
                                       OP.not_equal, OP.mult)
        # layout: partition p = q*4 + k (q = chunk-half 0..31, k = kb),
        # so chunk (ch, h)'s 4 kb rows are partitions 4*(2ch+h) .. +4
        xsgall = const.tile([128, 512], dt.bfloat16)
        nc.vector.tensor_tensor(xsgall[:], gm[:], xsb[:], OP.mult)
        for w in range(4):
            nc.sync.dma_start(xsgd.ap()[32 * w:32 * (w + 1), :],
                              xsgall[32 * w:32 * (w + 1), :])

        wdeq_t = [const.tile([128, 4 * 512], dt.bfloat16, name=f"wdeq{e}")
                  for e in range(EL)]

        def phase_b(e, gs=(0, 1, 2, 3)):
            wdv = wdeq_t[e][:].rearrange("p (g n) -> p g n", g=4)
            for g in gs:
                wr = wraw_p.tile([128, 512], dt.int8)
                nc.sync.dma_start(wr[:], wq.ap()[e, g])
                for nb in range(4):
                    col = e * 8 + (g // 2) * 4 + nb
                    nc.scalar.mul(wdv[:, g, nb * 128:(nb + 1) * 128],
                                  wr[:, nb * 128:(nb + 1) * 128],
                                  wsc[:, col:col + 1])

        phase_b(0, (0,))

        # ---- software-pipelined main loop: S-stage runs LA chunks ahead
        LA = 3
        xdqs = {}

        def s_stage(ch):
            gt = gt_p.tile([128, 512], dt.bfloat16)
            for r in range(4):
                q0 = 4 * (ch * 2 + r % 2)
                nc.sync.dma_start(gt[32 * r:32 * r + 4, :],
                                  xsgd.ap()[q0:q0 + 4, :])
            Xg = gat_p.tile([128, 2, 2048], dt.int8)
            nc.sync.dma_start(Xg[:], xg.ap()[ch])
            S = ssb_p.tile([128, 2 * R], dt.bfloat16)
            sp = [sps_p.tile([128, 512], dt.float32, name=f"sp{r}")
                  for r in range(4)]
            for r in range(4):
                c = r // 2
                nc.tensor.matmul(
                    sp[r][:],
                    msl[32 * r:32 * r + 4, c * 128:(c + 1) * 128],
                    gt[32 * r:32 * r + 4, :],
                    start=True, stop=True, tile_position=(32 * r, 0))
            for r in range(4):
                nc.scalar.copy(S[:, r * 512:(r + 1) * 512], sp[r][:])
            xdq = xdq_p.tile([128, 2, 2048], dt.bfloat16)
            for c in range(2):
                nc.vector.scalar_tensor_tensor(
                    xdq[:, c].rearrange("p (i b) -> p i b", b=2),
                    Xg[:, c].rearrange("p (i b) -> p i b", b=2), 1.0,
                    S[:, c * R:(c + 1) * R].unsqueeze(2)
                        .broadcast_to([128, R, 2]),
                    OP.mult, OP.mult)
            return xdq

        def main_stage(ch, xdq):
            e = ch // (4096 // R)
            ops = ops_p.tile([128, 512], dt.float32)
            wdv = wdeq_t[e][:].rearrange("p (g n) -> p g n", g=4)
            xv = xdq[:].rearrange("p c (t j b) -> p c b j t", t=128, j=8)
            first = True
            for c in range(2):
                for b in range(2):
                    g = 2 * c + b
                    for j in range(8):
                        nc.tensor.matmul(ops[:], xv[:, c, b, j, :],
                                         wdv[:, g, :],
                                         start=first,
                                         stop=(c == 1 and b == 1 and j == 7))
                        first = False
            sh = sh_p.tile([128, 512], dt.bfloat16)
            nc.sync.dma_start(sh[:], shared.ap()[ch * 128:(ch + 1) * 128, :])
            ob = ob_p.tile([128, 512], dt.bfloat16)
            nc.vector.scalar_tensor_tensor(ob[:], ops[:], 1.0, sh[:],
                                           OP.mult, OP.add)
            nc.sync.dma_start(out.ap()[ch * 128:(ch + 1) * 128, :], ob[:])

        for ch in range(n_chunks + LA):
            if ch == 1:
                phase_b(0, (1, 2, 3))
            if ch >= 4 and ch % 4 == 0 and ch // 4 < EL:
                phase_b(ch // 4)
            if ch < n_chunks:
                xdqs[ch] = s_stage(ch)
            if ch >= LA:
                main_stage(ch - LA, xdqs.pop(ch - LA))

    nc.compile()
    return nc


def _prep_inputs(input, weight, top_k_gates, token_indices, src_to_dst,
                 token_count, shared_output, weight_scale, input_scale):
    bf16 = ml_dtypes.bfloat16
    x = np.ascontiguousarray(np.asarray(input, dtype=np.int8))
    w = np.asarray(weight, dtype=np.int8)
    tkg = np.asarray(top_k_gates, dtype=np.float32)
    ti = np.asarray(token_indices, dtype=np.int32)
    s2d = np.asarray(src_to_dst, dtype=np.int32)
    sho = np.asarray(shared_output).astype(bf16)
    wsc = np.asarray(weight_scale, dtype=np.float32)
    xsc = np.asarray(input_scale, dtype=np.float32)

    p = np.arange(128)
    g = np.arange(4)
    kperm = 256 * (g[:, None] // 2) + 2 * p[None, :] + (g[:, None] % 2)  # [4,128]

    mselh = np.zeros((4, 256), bf16)
    for c in range(2):
        for pp in range(128):
            mselh[2 * c + pp // 64, c * 128 + pp] = 1.0
    mselrep = np.zeros((128, 256), bf16)
    for r in range(4):
        mselrep[32 * r:32 * r + 4, :] = mselh

    in_maps = []
    for cid in range(NCORES):
        e0 = cid * EL
        t0 = cid * TPC
        tl = ti[cid * RPC:(cid + 1) * RPC]
        # dispatch: pack this core's routed rows, pre-transposed
        xr = x[tl]                                   # [RPC, 512]
        arr = xr.reshape(NCH, R, 2, 128, 2)          # [ch, i, c, p, b]
        xgh = np.ascontiguousarray(
            np.transpose(arr, (0, 3, 2, 1, 4))).reshape(NCH, 128, 2, 2048)
        # q-major layout: partition p = q*4 + k holds [kb=k, rows q*512:+512)
        xs_rows = xsc[tl].astype(bf16)               # [RPC, 4]
        xsqh = np.ascontiguousarray(
            xs_rows.reshape(32, 512, 4).transpose(0, 2, 1)).reshape(128, 512)
        gfl = tkg[t0:t0 + TPC].reshape(-1)
        gtsh = np.ascontiguousarray(np.broadcast_to(
            gfl.reshape(32, 1, 512), (32, 4, 512))).reshape(128, 512)
        sfl = s2d[t0:t0 + TPC].reshape(-1)
        ssbh = np.ascontiguousarray(np.broadcast_to(
            sfl.reshape(32, 1, 512), (32, 4, 512))).reshape(128, 512)
        wq_h = np.ascontiguousarray(w[e0:e0 + EL][:, kperm, :])  # [EL,4,128,512]
        wcol = np.zeros((128, EL * 8), np.float32)
        for e in range(EL):
            for c in range(2):
                for nb in range(4):
                    wcol[:, e * 8 + c * 4 + nb] = wsc[e0 + e, 2 * c + p // 64, nb]
        mwb = np.zeros((128, 640), np.int8)
        mwb[:, 0:512] = mselrep.view(np.int8).reshape(128, 512)
        mwb[:, 512:640] = wcol.view(np.int8).reshape(128, 128)
        in_maps.append({
            "xg": xgh,
            "wq": wq_h,
            "gates": gtsh,
            "srcdst": ssbh,
            "xsq": xsqh,
            "mw": mwb,
            "shared": np.ascontiguousarray(sho[t0:t0 + TPC]),
        })
    return in_maps


def kernel(**inputs):
    from concourse import bass_utils
    if "nc" not in _cache:
        _cache["nc"] = _build()
    nc = _cache["nc"]
    in_maps = _prep_inputs(**inputs)
    import os
    res = bass_utils.run_bass_kernel_spmd(
        nc, in_maps, core_ids=list(range(NCORES)),
        trace=os.environ.get("BASS_TRACE") == "1")
    _cache["last_results"] = res
    out = np.concatenate([res.results[c]["out"] for c in range(NCORES)], axis=0)
    return out


# revision 39
# speedup vs baseline: 1.2055x; 1.2055x over previous
"""MoE grouped w8a8 block-quant GEMM + gated combine for 8 Trainium2 cores.

Sharding (expert-parallel, per the hint): core c owns experts [4c,4c+4),
their routed rows [c*16384,(c+1)*16384) (uniform token_count=4096), and
tokens [c*2048,(c+1)*2048). Routed rows are dispatched to their owning
core on the host (the all-to-all dispatch step): x rows are packed
pre-transposed into a K-on-partitions layout, so the device reads them
with fat sequential DMAs. All arithmetic (gate normalization/masking,
scale products, dequant, GEMM, combine) runs on device.

Device pipeline per core:
  Phase A (once): normalize+mask gates and multiply by gathered x-scales
    in a kb-major replicated [128, 512] layout -> xsg written to DRAM
    as a flat [4, RPC] table (row kb holds xs[row,kb]*gate[row]).
  Phase B (once): dequant int8 weights -> bf16 on the scalar engine.
  Main loop, per chunk of R=1024 routed rows (= 128 tokens):
    1. Sequential DMA pulls the pre-dispatched x chunk [128, 2, 2048] int8.
    2. A replicated-read DMA broadcasts xsg rows to all 128 partitions
       (S[p, c, i] = xsg[2c + p//64, ch*R+i]) -- no matmul, no PSUM.
    3. One DVE op dequantizes: xdq = int8 * S (bf16).
    4. 32 matmuls accumulate the gated expert GEMM into [128 tokens, 512]
       PSUM (gates are folded into S, so top-k combine == accumulation).
    5. DVE adds shared_output, writes bf16; DMA out.
"""

import numpy as np
import ml_dtypes

T, TOPK, K, N, E, B = 16384, 8, 512, 512, 32, 128
ROWS = T * TOPK
NCORES = 8
EL = E // NCORES            # experts per core
RPC = ROWS // NCORES        # routed rows per core
TPC = T // NCORES           # tokens per core
R = 1024                    # rows per chunk
NCH = RPC // R              # chunks per core

_cache = {}


def _build(n_chunks=NCH):
    from contextlib import ExitStack
    import concourse.bass as bass
    import concourse.bacc as bacc
    import concourse.tile as tile
    from concourse import mybir

    dt = mybir.dt
    nc = bacc.Bacc("TRN2", target_bir_lowering=False, debug=False,
                   enable_asserts=False)

    xg = nc.dram_tensor("xg", (NCH, 128, 2, 2048), dt.int8, kind="ExternalInput")
    wq = nc.dram_tensor("wq", (EL, 4, 128, 512), dt.int8, kind="ExternalInput")
    gates = nc.dram_tensor("gates", (128, 512), dt.float32, kind="ExternalInput")
    srcdst = nc.dram_tensor("srcdst", (128, 512), dt.int32, kind="ExternalInput")
    xsq = nc.dram_tensor("xsq", (128, 512), dt.bfloat16, kind="ExternalInput")
    # mw bytes: [0:512) msl bf16 (row-tile replicated), [512:640) wscol f32
    mw = nc.dram_tensor("mw", (128, 640), dt.int8, kind="ExternalInput")
    shared = nc.dram_tensor("shared", (TPC, N), dt.bfloat16, kind="ExternalInput")
    out = nc.dram_tensor("out", (TPC, N), dt.bfloat16, kind="ExternalOutput")
    xsgd = nc.dram_tensor("xsgd", (128, 512), dt.bfloat16, kind="Internal")

    AX = mybir.AxisListType
    OP = mybir.AluOpType

    with tile.TileContext(nc) as tc, ExitStack() as ctx:
        const = ctx.enter_context(tc.tile_pool(name="const", bufs=1))
        wraw_p = ctx.enter_context(tc.tile_pool(name="wraw", bufs=2))
        gat_p = ctx.enter_context(tc.tile_pool(name="gat", bufs=5))
        gt_p = ctx.enter_context(tc.tile_pool(name="gtp", bufs=5))
        ssb_p = ctx.enter_context(tc.tile_pool(name="ssb", bufs=4))
        xdq_p = ctx.enter_context(tc.tile_pool(name="xdq", bufs=4))
        sh_p = ctx.enter_context(tc.tile_pool(name="shp", bufs=3))
        ob_p = ctx.enter_context(tc.tile_pool(name="obp", bufs=3))
        sps_p = ctx.enter_context(tc.tile_pool(name="sps", bufs=1, space="PSUM"))
        ops_p = ctx.enter_context(tc.tile_pool(name="ops", bufs=2, space="PSUM"))

        # ---- phase A: xsg = normalized+masked gates * x-scales (kb-major)
        gsb = const.tile([128, 512], dt.float32)
        nc.sync.dma_start(gsb[:], gates.ap())
        ssb = const.tile([128, 512], dt.int32)
        nc.sync.dma_start(ssb[:], srcdst.ap())
        xsb = const.tile([128, 512], dt.bfloat16)
        nc.sync.dma_start(xsb[:], xsq.ap())
        mwt = const.tile([128, 640], dt.int8)
        nc.sync.dma_start(mwt[:], mw.ap())
        msl = mwt[:].bitcast(dt.bfloat16)[:, 0:256]
        wsc = mwt[:].bitcast(dt.float32)[:, 128:160]
        g3 = gsb[:].rearrange("p (t j) -> p t j", j=8)
        sums = const.tile([128, 64], dt.float32)
        nc.vector.tensor_reduce(sums[:], g3, AX.X, OP.add)
        nc.vector.tensor_scalar_max(sums[:], sums[:], 1e-12)
        rec = const.tile([128, 64], dt.float32)
        nc.vector.reciprocal(rec[:], sums[:])
        gn = const.tile([128, 512], dt.float32)
        nc.vector.scalar_tensor_tensor(
            gn[:].rearrange("p (t j) -> p t j", j=8), g3, 1.0,
            rec[:].unsqueeze(2).broadcast_to([128, 64, 8]), OP.mult, OP.mult)
        gm = const.tile([128, 512], dt.bfloat16)
        nc.vector.scalar_tensor_tensor(gm[:], ssb[:], -1, gn[:],
                                       OP.not_equal, OP.mult)
        # layout: partition p = q*4 + k (q = chunk-half 0..31, k = kb),
        # so chunk (ch, h)'s 4 kb rows are partitions 4*(2ch+h) .. +4
        xsgall = const.tile([128, 512], dt.bfloat16)
        nc.vector.tensor_tensor(xsgall[:], gm[:], xsb[:], OP.mult)
        for w in range(4):
            nc.sync.dma_start(xsgd.ap()[32 * w:32 * (w + 1), :],
                              xsgall[32 * w:32 * (w + 1), :])

        wdeq_t = [const.tile([128, 4 * 512], dt.bfloat16, name=f"wdeq{e}")
                  for e in range(EL)]

        def phase_b(e, gs=(0, 1, 2, 3)):
            wdv = wdeq_t[e][:].rearrange("p (g n) -> p g n", g=4)
            for g in gs:
                wr = wraw_p.tile([128, 512], dt.int8)
                nc.sync.dma_start(wr[:], wq.ap()[e, g])
                for nb in range(4):
                    col = e * 8 + (g // 2) * 4 + nb
                    nc.scalar.mul(wdv[:, g, nb * 128:(nb + 1) * 128],
                                  wr[:, nb * 128:(nb + 1) * 128],
                                  wsc[:, col:col + 1])

        phase_b(0, (0,))

        # ---- software-pipelined main loop: S-stage runs LA chunks ahead
        LA = 2
        xdqs = {}

        def s_stage(ch):
            gt = gt_p.tile([128, 512], dt.bfloat16)
            for r in range(4):
                q0 = 4 * (ch * 2 + r % 2)
                nc.sync.dma_start(gt[32 * r:32 * r + 4, :],
                                  xsgd.ap()[q0:q0 + 4, :])
            Xg = gat_p.tile([128, 2, 2048], dt.int8)
            nc.sync.dma_start(Xg[:], xg.ap()[ch])
            S = ssb_p.tile([128, 2 * R], dt.bfloat16)
            sp = [sps_p.tile([128, 512], dt.float32, name=f"sp{r}")
                  for r in range(4)]
            for r in range(4):
                c = r // 2
                nc.tensor.matmul(
                    sp[r][:],
                    msl[32 * r:32 * r + 4, c * 128:(c + 1) * 128],
                    gt[32 * r:32 * r + 4, :],
                    start=True, stop=True, tile_position=(32 * r, 0))
            for r in range(4):
                nc.scalar.copy(S[:, r * 512:(r + 1) * 512], sp[r][:])
            xdq = xdq_p.tile([128, 2, 2048], dt.bfloat16)
            for c in range(2):
                nc.vector.scalar_tensor_tensor(
                    xdq[:, c].rearrange("p (i b) -> p i b", b=2),
                    Xg[:, c].rearrange("p (i b) -> p i b", b=2), 1.0,
                    S[:, c * R:(c + 1) * R].unsqueeze(2)
                        .broadcast_to([128, R, 2]),
                    OP.mult, OP.mult)
            return xdq

        def main_stage(ch, xdq):
            e = ch // (4096 // R)
            ops = ops_p.tile([128, 512], dt.float32)
            wdv = wdeq_t[e][:].rearrange("p (g n) -> p g n", g=4)
            xv = xdq[:].rearrange("p c (t j b) -> p c b j t", t=128, j=8)
            first = True
            for c in range(2):
                for b in range(2):
                    g = 2 * c + b
                    for j in range(8):
                        nc.tensor.matmul(ops[:], xv[:, c, b, j, :],
                                         wdv[:, g, :],
                                         start=first,
                                         stop=(c == 1 and b == 1 and j == 7))
                        first = False
            sh = sh_p.tile([128, 512], dt.bfloat16)
            nc.sync.dma_start(sh[:], shared.ap()[ch * 128:(ch + 1) * 128, :])
            ob = ob_p.tile([128, 512], dt.bfloat16)
            nc.vector.scalar_tensor_tensor(ob[:], ops[:], 1.0, sh[:],
                                           OP.mult, OP.add)
            nc.sync.dma_start(out.ap()[ch * 128:(ch + 1) * 128, :], ob[:])

        for ch in range(n_chunks + LA):
            if ch == 1:
                phase_b(0, (1, 2, 3))
            if ch >= 4 and ch % 4 == 0 and ch // 4 < EL:
                phase_b(ch // 4)
            if ch < n_chunks:
                xdqs[ch] = s_stage(ch)
            if ch >= LA:
                main_stage(ch - LA, xdqs.pop(ch - LA))

    nc.compile()
    return nc


def _prep_inputs(input, weight, top_k_gates, token_indices, src_to_dst,
                 token_count, shared_output, weight_scale, input_scale):
    bf16 = ml_dtypes.bfloat16
    x = np.ascontiguousarray(np.asarray(input, dtype=np.int8))
    w = np.asarray(weight, dtype=np.int8)
    tkg = np.asarray(top_k_gates, dtype=np.float32)
    ti = np.asarray(token_indices, dtype=np.int32)
    s2d = np.asarray(src_to_dst, dtype=np.int32)
    sho = np.asarray(shared_output).astype(bf16)
    wsc = np.asarray(weight_scale, dtype=np.float32)
    xsc = np.asarray(input_scale, dtype=np.float32)

    p = np.arange(128)
    g = np.arange(4)
    kperm = 256 * (g[:, None] // 2) + 2 * p[None, :] + (g[:, None] % 2)  # [4,128]

    mselh = np.zeros((4, 256), bf16)
    for c in range(2):
        for pp in range(128):
            mselh[2 * c + pp // 64, c * 128 + pp] = 1.0
    mselrep = np.zeros((128, 256), bf16)
    for r in range(4):
        mselrep[32 * r:32 * r + 4, :] = mselh

    in_maps = []
    for cid in range(NCORES):
        e0 = cid * EL
        t0 = cid * TPC
        tl = ti[cid * RPC:(cid + 1) * RPC]
        # dispatch: pack this core's routed rows, pre-transposed
        xr = x[tl]                                   # [RPC, 512]
        arr = xr.reshape(NCH, R, 2, 128, 2)          # [ch, i, c, p, b]
        xgh = np.ascontiguousarray(
            np.transpose(arr, (0, 3, 2, 1, 4))).reshape(NCH, 128, 2, 2048)
        # q-major layout: partition p = q*4 + k holds [kb=k, rows q*512:+512)
        xs_rows = xsc[tl].astype(bf16)               # [RPC, 4]
        xsqh = np.ascontiguousarray(
            xs_rows.reshape(32, 512, 4).transpose(0, 2, 1)).reshape(128, 512)
        gfl = tkg[t0:t0 + TPC].reshape(-1)
        gtsh = np.ascontiguousarray(np.broadcast_to(
            gfl.reshape(32, 1, 512), (32, 4, 512))).reshape(128, 512)
        sfl = s2d[t0:t0 + TPC].reshape(-1)
        ssbh = np.ascontiguousarray(np.broadcast_to(
            sfl.reshape(32, 1, 512), (32, 4, 512))).reshape(128, 512)
        wq_h = np.ascontiguousarray(w[e0:e0 + EL][:, kperm, :])  # [EL,4,128,512]
        wcol = np.zeros((128, EL * 8), np.float32)
        for e in range(EL):
            for c in range(2):
                for nb in range(4):
                    wcol[:, e * 8 + c * 4 + nb] = wsc[e0 + e, 2 * c + p // 64, nb]
        mwb = np.zeros((128, 640), np.int8)
        mwb[:, 0:512] = mselrep.view(np.int8).reshape(128, 512)
        mwb[:, 512:640] = wcol.view(np.int8).reshape(128, 128)
        in_maps.append({
            "xg": xgh,
            "wq": wq_h,
            "gates": gtsh,
            "srcdst": ssbh,
            "xsq": xsqh,
            "mw": mwb,
            "shared": np.ascontiguousarray(sho[t0:t0 + TPC]),
        })
    return in_maps


def kernel(**inputs):
    from concourse import bass_utils
    if "nc" not in _cache:
        _cache["nc"] = _build()
    nc = _cache["nc"]
    in_maps = _prep_inputs(**inputs)
    import os
    res = bass_utils.run_bass_kernel_spmd(
        nc, in_maps, core_ids=list(range(NCORES)),
        trace=os.environ.get("BASS_TRACE") == "1")
    _cache["last_results"] = res
    out = np.concatenate([res.results[c]["out"] for c in range(NCORES)], axis=0)
    return out
